# revision 2
# baseline (speedup 1.0000x reference)
"""CoralLoss TRN2 kernel v5: v3 structure + ramp/tail chunk splitting.

8-core SPMD, 512 tokens/core, groups of 128 tokens. Vocab processed in
per-group chunk lists: first group starts with 2x4000 chunks (compute
starts ~6us earlier), last group ends with 2x4000 (shorter tail after
the final DMA); middle chunks are 8000 wide.

Per chunk (width w, S_w = DVE relu cols):
  DVE: min-plain (4x) -> mt; relu+accum on [:S_w]; TT max-tree w->1000
       + max-accum(1000) -> chunk max
  ACT: recip(1-mt)+accum -> sum_recip; relu+accum on [S_w:]
Host: sum_s = sum_recip + sum_relu(act) + sum_relu(dve);
      correct <=> tgt_bf16 >= max over chunks.
"""

import os
import ml_dtypes
import numpy as np
from contextlib import ExitStack

import concourse.bass as bass
import concourse.tile as tile
from concourse import bacc, mybir
from concourse.bass_utils import run_bass_kernel_spmd

B, L, V = 4, 1024, 32000
N_CORES = 8
TOK = B * L
TPC = TOK // N_CORES      # 512
P = 128
G = TPC // P              # 4
S8 = int(os.environ.get("K5_S", "3712"))   # DVE relu cols per 8000
XBUFS = int(os.environ.get("K5_XBUFS", "4"))
CMAX = 5                  # max chunks per group
IGNORE_LABEL_ID = -100


def _chunks_for_group(g):
    if g == 0:
        ws = [4000, 4000, 8000, 8000, 8000]
    elif g == G - 1:
        ws = [8000, 8000, 8000, 4000, 4000]
    else:
        ws = [8000, 8000, 8000, 8000]
    out, a = [], 0
    for w in ws:
        out.append((a, w))
        a += w
    assert a == V
    return out


CHUNKS = [_chunks_for_group(g) for g in range(G)]

_NC_CACHE = {}


def _raw_activation(eng, out, in_, func, bias=0.0, scale=1.0, accum_out=None):
    b = eng.bass
    if func not in (
        mybir.ActivationFunctionType.Copy,
        mybir.ActivationFunctionType.Reciprocal,
    ) and isinstance(bias, float):
        bias = b.const_aps.scalar_like(bias, in_)
    inputs = [eng.lower_ap(in_)]
    for arg in (bias, scale, 0.0):
        if isinstance(arg, bass.AP):
            inputs.append(eng.lower_ap(arg))
        else:
            inputs.append(mybir.ImmediateValue(dtype=mybir.dt.float32, value=arg))
    outputs = [eng.lower_ap(out)]
    if accum_out is not None:
        outputs.append(eng.lower_ap(accum_out))
    return eng.add_instruction(
        mybir.InstActivation(
            name=b.get_next_instruction_name(), func=func, ins=inputs, outs=outputs
        )
    )


def _s_for(w):
    s = (w * S8 // 8000) & ~15
    return s


def _build():
    if "nc" in _NC_CACHE:
        return _NC_CACHE["nc"]
    nc = bacc.Bacc("TRN2", debug=False, target_bir_lowering=False)
    f32 = mybir.dt.float32
    bf16 = mybir.dt.bfloat16
    Recip = mybir.ActivationFunctionType.Reciprocal
    Relu = mybir.ActivationFunctionType.Relu
    Alu = mybir.AluOpType
    F = 8000

    x = nc.dram_tensor("x", [TPC, V], f32, kind="ExternalInput").ap()
    # out[g, :, 0:CMAX]=sum_recip  CMAX:2C=relu(ACT)  2C:3C=chunk max
    # 3C:4C=relu(DVE); cols beyond ncg per group are unwritten garbage
    out = nc.dram_tensor("out", [G, P, 4 * CMAX], f32, kind="ExternalOutput").ap()

    xv = x.rearrange("(g p) v -> g p v", p=P)

    with tile.TileContext(nc) as tc, ExitStack() as ctx:
        xpool = ctx.enter_context(tc.tile_pool(name="x", bufs=XBUFS))
        mpool = ctx.enter_context(tc.tile_pool(name="m", bufs=3))
        spool = ctx.enter_context(tc.tile_pool(name="scr", bufs=1))
        apool = ctx.enter_context(tc.tile_pool(name="acc", bufs=1))

        smax = _s_for(F)
        scr_dve = spool.tile([P, smax], bf16, tag="scr_dve")
        scr_act = spool.tile([P, F - smax], bf16, tag="scr_act")
        scr_r = spool.tile([P, F], bf16, tag="scr_r")
        t1 = spool.tile([P, F // 2], bf16, tag="t1")
        t2 = spool.tile([P, F // 4], bf16, tag="t2")
        t3 = spool.tile([P, F // 8], bf16, tag="t3")
        t3s = spool.tile([P, F // 8], bf16, tag="t3s")

        for g in range(G):
            chunks = CHUNKS[g]
            acc_act = apool.tile([P, 2 * CMAX], f32, tag=f"acc_act{g}")
            acc_dve = apool.tile([P, 2 * CMAX], f32, tag=f"acc_dve{g}")
            for j, (a, w) in enumerate(chunks):
                s = _s_for(w)
                xt = xpool.tile([P, w], bf16)
                nc.gpsimd.dma_start(xt, xv[g, :, a:a + w])

                mt = mpool.tile([P, w], bf16)
                nc.vector.tensor_scalar(
                    out=mt, in0=xt, scalar1=0.0, scalar2=None, op0=Alu.min,
                )
                _raw_activation(
                    nc.scalar, scr_r[:, :w], mt, Recip, bias=1.0, scale=-1.0,
                    accum_out=acc_act[:, j:j + 1],
                )
                _raw_activation(
                    nc.scalar, scr_act[:, :w - s], xt[:, s:], Relu,
                    accum_out=acc_act[:, CMAX + j:CMAX + j + 1],
                )
                nc.vector.tensor_scalar(
                    out=scr_dve[:, :s], in0=xt[:, :s], scalar1=0.0,
                    scalar2=None, op0=Alu.max, op1=Alu.add,
                    accum_out=acc_dve[:, CMAX + j:CMAX + j + 1],
                )
                # max tree w -> 1000 then max-accum
                lv = w
                src = xt
                bufs = [t1, t2, t3]
                bi = 0 if w == F else 1
                while lv > 1000:
                    dst = bufs[bi]
                    nc.vector.tensor_tensor(
                        out=dst[:, :lv // 2], in0=src[:, :lv // 2],
                        in1=src[:, lv // 2:lv], op=Alu.max)
                    src = dst
                    lv //= 2
                    bi += 1
                nc.vector.tensor_scalar(
                    out=t3s, in0=src[:, :1000], scalar1=-3.0e38, scalar2=None,
                    op0=Alu.max, op1=Alu.max,
                    accum_out=acc_dve[:, j:j + 1],
                )
            ncg = len(chunks)
            nc.sync.dma_start(out[g, :, 0:ncg], acc_act[:, 0:ncg])
            nc.sync.dma_start(
                out[g, :, CMAX:CMAX + ncg], acc_act[:, CMAX:CMAX + ncg])
            nc.sync.dma_start(
                out[g, :, 2 * CMAX:2 * CMAX + ncg], acc_dve[:, 0:ncg])
            nc.sync.dma_start(
                out[g, :, 3 * CMAX:3 * CMAX + ncg],
                acc_dve[:, CMAX:CMAX + ncg])

    nc.compile()
    _NC_CACHE["nc"] = nc
    return nc


def _run_device(flat_logits, trace=False):
    nc = _build()
    in_maps = []
    for c in range(N_CORES):
        xs = np.ascontiguousarray(flat_logits[c * TPC:(c + 1) * TPC])
        in_maps.append({"x": xs})
    res = run_bass_kernel_spmd(
        nc, in_maps, core_ids=list(range(N_CORES)), trace=trace
    )
    sum_s = np.empty(TOK, np.float64)
    mx = np.empty(TOK, np.float64)
    for c, r in enumerate(res.results):
        o = r["out"].astype(np.float64)  # [G, P, 4*CMAX]
        for g in range(G):
            ncg = len(CHUNKS[g])
            s = (o[g, :, 0:ncg].sum(-1)
                 + o[g, :, CMAX:CMAX + ncg].sum(-1)
                 + o[g, :, 3 * CMAX:3 * CMAX + ncg].sum(-1))  # [P]
            k = o[g, :, 2 * CMAX:2 * CMAX + ncg].max(-1)
            base = c * TPC + g * P
            sum_s[base:base + P] = s
            mx[base:base + P] = k
    return sum_s, mx, res


def _bce_with_logits(x, t):
    return np.mean(np.maximum(x, 0.0) - x * t + np.log1p(np.exp(-np.abs(x))))


def kernel(logits, q_halt_logits, q_continue_logits, labels, _trace=False,
           _return_res=False):
    assert logits.shape == (B, L, V), logits.shape
    logits = np.asarray(logits, dtype=np.float32)
    labels = np.asarray(labels)
    qh = np.asarray(q_halt_logits, dtype=np.float64)
    qc = np.asarray(q_continue_logits, dtype=np.float64)

    valid = labels != IGNORE_LABEL_ID
    safe = np.where(valid, labels, 0).astype(np.int64)
    flat = logits.reshape(TOK, V)
    tgt_full = flat[np.arange(TOK), safe.reshape(-1)].astype(np.float32)

    sum_s, mx, res = _run_device(flat, trace=_trace)

    x_t = tgt_full.astype(np.float64)
    s_t = np.where(x_t >= 0, x_t + 1.0, 1.0 / (1.0 - x_t + 1e-30))
    per_token = np.log(sum_s) - np.log(s_t)
    per_token = np.where(valid.reshape(-1), per_token, 0.0).reshape(B, L)

    loss_counts = np.maximum(valid.sum(-1), 1).astype(np.float64)
    l_task = np.mean(per_token.sum(-1) / loss_counts)

    tgt_b = tgt_full.astype(ml_dtypes.bfloat16).astype(np.float64)
    correct = (tgt_b >= mx) & valid.reshape(-1)
    correct = correct.reshape(B, L)
    seq_correct = correct.sum(-1) == valid.sum(-1)
    halt_target = seq_correct.astype(np.float64)
    l_halt = _bce_with_logits(qh, halt_target)
    target_continue = 1.0 / (1.0 + np.exp(-qh))
    l_halt = 0.5 * (l_halt + _bce_with_logits(qc, target_continue))

    total = np.array(l_task + l_halt, dtype=np.float32)
    if _return_res:
        return total, res
    return total


# revision 3
# speedup vs baseline: 1.0014x; 1.0014x over previous
"""CoralLoss TRN2 kernel v5: v3 structure + ramp/tail chunk splitting.

8-core SPMD, 512 tokens/core, groups of 128 tokens. Vocab processed in
per-group chunk lists: first group starts with 2x4000 chunks (compute
starts ~6us earlier), last group ends with 2x4000 (shorter tail after
the final DMA); middle chunks are 8000 wide.

Per chunk (width w, S_w = DVE relu cols):
  DVE: min-plain (4x) -> mt; relu+accum on [:S_w]; TT max-tree w->1000
       + max-accum(1000) -> chunk max
  ACT: recip(1-mt)+accum -> sum_recip; relu+accum on [S_w:]
Host: sum_s = sum_recip + sum_relu(act) + sum_relu(dve);
      correct <=> tgt_bf16 >= max over chunks.
"""

import os
import ml_dtypes
import numpy as np
from contextlib import ExitStack

import concourse.bass as bass
import concourse.tile as tile
from concourse import bacc, mybir
from concourse.bass_utils import run_bass_kernel_spmd

B, L, V = 4, 1024, 32000
N_CORES = 8
TOK = B * L
TPC = TOK // N_CORES      # 512
P = 128
G = TPC // P              # 4
S8 = int(os.environ.get("K5_S", "3712"))   # DVE relu cols per 8000
XBUFS = int(os.environ.get("K5_XBUFS", "4"))
MBUFS = int(os.environ.get("K5_MBUFS", "3"))
CMAX = 5                  # max chunks per group
IGNORE_LABEL_ID = -100


def _chunks_for_group(g):
    if g == 0:
        ws = [4000, 4000, 8000, 8000, 8000]
    elif g == G - 1:
        ws = [8000, 8000, 8000, 4000, 4000]
    else:
        ws = [8000, 8000, 8000, 8000]
    out, a = [], 0
    for w in ws:
        out.append((a, w))
        a += w
    assert a == V
    return out


CHUNKS = [_chunks_for_group(g) for g in range(G)]

_NC_CACHE = {}


def _raw_activation(eng, out, in_, func, bias=0.0, scale=1.0, accum_out=None):
    b = eng.bass
    if func not in (
        mybir.ActivationFunctionType.Copy,
        mybir.ActivationFunctionType.Reciprocal,
    ) and isinstance(bias, float):
        bias = b.const_aps.scalar_like(bias, in_)
    inputs = [eng.lower_ap(in_)]
    for arg in (bias, scale, 0.0):
        if isinstance(arg, bass.AP):
            inputs.append(eng.lower_ap(arg))
        else:
            inputs.append(mybir.ImmediateValue(dtype=mybir.dt.float32, value=arg))
    outputs = [eng.lower_ap(out)]
    if accum_out is not None:
        outputs.append(eng.lower_ap(accum_out))
    return eng.add_instruction(
        mybir.InstActivation(
            name=b.get_next_instruction_name(), func=func, ins=inputs, outs=outputs
        )
    )


def _s_for(w):
    s = (w * S8 // 8000) & ~15
    return s


def _build():
    if "nc" in _NC_CACHE:
        return _NC_CACHE["nc"]
    nc = bacc.Bacc("TRN2", debug=False, target_bir_lowering=False)
    f32 = mybir.dt.float32
    bf16 = mybir.dt.bfloat16
    Recip = mybir.ActivationFunctionType.Reciprocal
    Relu = mybir.ActivationFunctionType.Relu
    Alu = mybir.AluOpType
    F = 8000

    x = nc.dram_tensor("x", [TPC, V], f32, kind="ExternalInput").ap()
    # out[g, :, 0:CMAX]=sum_recip  CMAX:2C=relu(ACT)  2C:3C=chunk max
    # 3C:4C=relu(DVE); cols beyond ncg per group are unwritten garbage
    out = nc.dram_tensor("out", [G, P, 4 * CMAX], f32, kind="ExternalOutput").ap()

    xv = x.rearrange("(g p) v -> g p v", p=P)

    with tile.TileContext(nc) as tc, ExitStack() as ctx:
        xpool = ctx.enter_context(tc.tile_pool(name="x", bufs=XBUFS))
        mpool = ctx.enter_context(tc.tile_pool(name="m", bufs=MBUFS))
        spool = ctx.enter_context(tc.tile_pool(name="scr", bufs=1))
        apool = ctx.enter_context(tc.tile_pool(name="acc", bufs=1))

        smax = _s_for(F)
        scr_dve = spool.tile([P, smax], bf16, tag="scr_dve")
        scr_act = spool.tile([P, F - smax], bf16, tag="scr_act")
        scr_r = spool.tile([P, F], bf16, tag="scr_r")
        t1 = spool.tile([P, F // 2], bf16, tag="t1")
        t2 = spool.tile([P, F // 4], bf16, tag="t2")
        t3 = spool.tile([P, F // 8], bf16, tag="t3")
        t3s = spool.tile([P, F // 8], bf16, tag="t3s")

        for g in range(G):
            chunks = CHUNKS[g]
            acc_act = apool.tile([P, 2 * CMAX], f32, tag=f"acc_act{g}")
            acc_dve = apool.tile([P, 2 * CMAX], f32, tag=f"acc_dve{g}")
            for j, (a, w) in enumerate(chunks):
                s = _s_for(w)
                xt = xpool.tile([P, w], bf16)
                nc.gpsimd.dma_start(xt, xv[g, :, a:a + w])

                mt = mpool.tile([P, w], bf16)
                nc.vector.tensor_scalar(
                    out=mt, in0=xt, scalar1=0.0, scalar2=None, op0=Alu.min,
                )
                _raw_activation(
                    nc.scalar, scr_r[:, :w], mt, Recip, bias=1.0, scale=-1.0,
                    accum_out=acc_act[:, j:j + 1],
                )
                _raw_activation(
                    nc.scalar, scr_act[:, :w - s], xt[:, s:], Relu,
                    accum_out=acc_act[:, CMAX + j:CMAX + j + 1],
                )
                nc.vector.tensor_scalar(
                    out=scr_dve[:, :s], in0=xt[:, :s], scalar1=0.0,
                    scalar2=None, op0=Alu.max, op1=Alu.add,
                    accum_out=acc_dve[:, CMAX + j:CMAX + j + 1],
                )
                # max tree w -> 1000 then max-accum
                lv = w
                src = xt
                bufs = [t1, t2, t3]
                bi = 0 if w == F else 1
                while lv > 1000:
                    dst = bufs[bi]
                    nc.vector.tensor_tensor(
                        out=dst[:, :lv // 2], in0=src[:, :lv // 2],
                        in1=src[:, lv // 2:lv], op=Alu.max)
                    src = dst
                    lv //= 2
                    bi += 1
                nc.vector.tensor_scalar(
                    out=t3s, in0=src[:, :1000], scalar1=-3.0e38, scalar2=None,
                    op0=Alu.max, op1=Alu.max,
                    accum_out=acc_dve[:, j:j + 1],
                )
            ncg = len(chunks)
            nc.sync.dma_start(out[g, :, 0:ncg], acc_act[:, 0:ncg])
            nc.sync.dma_start(
                out[g, :, CMAX:CMAX + ncg], acc_act[:, CMAX:CMAX + ncg])
            nc.sync.dma_start(
                out[g, :, 2 * CMAX:2 * CMAX + ncg], acc_dve[:, 0:ncg])
            nc.sync.dma_start(
                out[g, :, 3 * CMAX:3 * CMAX + ncg],
                acc_dve[:, CMAX:CMAX + ncg])

    nc.compile()
    _NC_CACHE["nc"] = nc
    return nc


def _run_device(flat_logits, trace=False):
    nc = _build()
    in_maps = []
    for c in range(N_CORES):
        xs = np.ascontiguousarray(flat_logits[c * TPC:(c + 1) * TPC])
        in_maps.append({"x": xs})
    res = run_bass_kernel_spmd(
        nc, in_maps, core_ids=list(range(N_CORES)), trace=trace
    )
    sum_s = np.empty(TOK, np.float64)
    mx = np.empty(TOK, np.float64)
    for c, r in enumerate(res.results):
        o = r["out"].astype(np.float64)  # [G, P, 4*CMAX]
        for g in range(G):
            ncg = len(CHUNKS[g])
            s = (o[g, :, 0:ncg].sum(-1)
                 + o[g, :, CMAX:CMAX + ncg].sum(-1)
                 + o[g, :, 3 * CMAX:3 * CMAX + ncg].sum(-1))  # [P]
            k = o[g, :, 2 * CMAX:2 * CMAX + ncg].max(-1)
            base = c * TPC + g * P
            sum_s[base:base + P] = s
            mx[base:base + P] = k
    return sum_s, mx, res


def _bce_with_logits(x, t):
    return np.mean(np.maximum(x, 0.0) - x * t + np.log1p(np.exp(-np.abs(x))))


def kernel(logits, q_halt_logits, q_continue_logits, labels, _trace=False,
           _return_res=False):
    assert logits.shape == (B, L, V), logits.shape
    logits = np.asarray(logits, dtype=np.float32)
    labels = np.asarray(labels)
    qh = np.asarray(q_halt_logits, dtype=np.float64)
    qc = np.asarray(q_continue_logits, dtype=np.float64)

    valid = labels != IGNORE_LABEL_ID
    safe = np.where(valid, labels, 0).astype(np.int64)
    flat = logits.reshape(TOK, V)
    tgt_full = flat[np.arange(TOK), safe.reshape(-1)].astype(np.float32)

    sum_s, mx, res = _run_device(flat, trace=_trace)

    x_t = tgt_full.astype(np.float64)
    s_t = np.where(x_t >= 0, x_t + 1.0, 1.0 / (1.0 - x_t + 1e-30))
    per_token = np.log(sum_s) - np.log(s_t)
    per_token = np.where(valid.reshape(-1), per_token, 0.0).reshape(B, L)

    loss_counts = np.maximum(valid.sum(-1), 1).astype(np.float64)
    l_task = np.mean(per_token.sum(-1) / loss_counts)

    tgt_b = tgt_full.astype(ml_dtypes.bfloat16).astype(np.float64)
    correct = (tgt_b >= mx) & valid.reshape(-1)
    correct = correct.reshape(B, L)
    seq_correct = correct.sum(-1) == valid.sum(-1)
    halt_target = seq_correct.astype(np.float64)
    l_halt = _bce_with_logits(qh, halt_target)
    target_continue = 1.0 / (1.0 + np.exp(-qh))
    l_halt = 0.5 * (l_halt + _bce_with_logits(qc, target_continue))

    total = np.array(l_task + l_halt, dtype=np.float32)
    if _return_res:
        return total, res
    return total
